# revision 28
# baseline (speedup 1.0000x reference)
"""ComplexMDTA Trainium2 kernel.

Sharding: 8 cores = (batch 4) x (H halves 2). Each core computes its
(batch, 96-row) slice end-to-end. The only cross-core data dependency
is the q/k L2-norm sums + q@k^T Gram matrices (reductions over the full
H*W axis), handled with a pairwise AllReduce between the two cores
sharing a batch, hidden behind the v convolution.

Per core:
  phase A (q,k): fused conv1x1+dwconv3x3 via fp8e4 DoubleRow matmuls.
        Host interleaves (xr, xi) pairs into one fp8 stream and packs
        the per-tap complex weights as DoubleRow pairs, so
        yr = Mr@xr + (-Mi)@xi and yi = Mi@xr + Mr@xi are each ONE
        DoubleRow matmul per tap (0.5 cyc/row), PSUM-accumulated over
        the 9 taps; the epilogue is a plain PSUM->bf16 copy (no Gauss
        combine). fp8 quantization noise averages out in the Gram
        (summed over 36864 px) and the weight scale S cancels in the
        L2 normalization. Tap-outer sweeps over chunk sets reuse one
        DoubleRow weight load (DoubleRow disables FWL, so LDWEIGHTS is
        expensive; _dedupe_ldweights drops the redundant reloads).
        q/k are transposed for the Gram with blocked DMA-XBAR
        transposes (one instruction per tensor per block).
  A->B: stage Gram+sumsq, launch pairwise AllReduce (gpsimd), which
        overlaps with phase B.
  phase B (v): bf16 Gauss conv writing v into SBUF-resident bf16
        tiles (no HBM roundtrip), also tap-outer over chunk pairs to
        share weight loads. The softmax/normalization (phase C1) is
        emitted between blocks 4 and 5 so it hides inside phase B.
  phase C2: fold the projection into the attention matrix on PE:
        FT = (A @ P)^T blocks.
  phase D: (P.A)@v from SBUF v tiles, FT-outer matmul order (3 weight
        loads per chunk), outputs DMAed on both hwdge queues.
"""
import os
import sys

for _p in ('/opt/trn_rl_repo', '/root/.axon_site/_ro/trn_rl_repo'):
    if os.path.isdir(_p) and _p not in sys.path:
        sys.path.insert(0, _p)

import numpy as np
import ml_dtypes
import concourse.bass as bass
import concourse.tile as tile
import concourse.mybir as mybir
from concourse.bass_utils import run_bass_kernel_spmd

dt = mybir.dt
F32 = dt.float32
BF16 = dt.bfloat16
F8 = dt.float8e4
ALU = mybir.AluOpType
AF = mybir.ActivationFunctionType
DR = mybir.MatmulPerfMode.DoubleRow

B, C, H, W = 4, 128, 192, 192
HEADS = 8
C3 = 3 * C
HH = H // 2          # rows per core
SLAB = HH + 2        # input rows incl halo
Wp = W + 2           # padded width
RB = 16              # output rows per block
NB = HH // RB        # blocks per core
NCHUNK = RB // 2     # 2-row chunks per block
CH_N = 2 * Wp        # matmul free size per chunk (388)
NPX = HH * W         # unpadded px per core (18432)
P4N = 8 * W          # phase-4 chunk px (1536)
NP4 = NPX // P4N     # 12
GN = 18 * Wp         # slab px per block
M8SCALE = 2048.0     # fp8 weight scale (cancels in the normalization)


def _split_multi_waits(nc, max_waits=1):
    # This walrus build rejects instructions carrying more than one sem
    # wait (and Drain carrying any); spill extras onto same-engine NoOps.
    ctr = 0
    for f in nc.m.functions:
        for bb in f.blocks:
            new = []
            changed = False
            for inst in bb.instructions:
                si = inst.sync_info
                nw = len(si.on_wait) if si is not None else 0
                limit = 0 if inst.opcode == "Drain" else max_waits
                if si is not None and nw > limit:
                    waits = list(si.on_wait)
                    keep = waits[nw - limit:] if limit else []
                    spill = waits[:nw - limit] if limit else waits
                    for w in spill:
                        ctr += 1
                        nop = mybir.InstNoOp(name=f"WSPLIT-{ctr}", ins=[], outs=[])
                        nop.engine = inst.engine
                        nop.sync_info = mybir.SyncInfo(on_wait=[w], on_update=[])
                        new.append(nop)
                    inst.sync_info = mybir.SyncInfo(
                        on_wait=keep, on_update=list(si.on_update))
                    changed = True
                new.append(inst)
            if changed:
                bb.instructions = new


def _dedupe_ldweights(nc):
    """Drop Ldweights whose weights AP is identical to the previous PE
    weight load with only Matmult/NoOp between (PE array keeps weights
    loaded); merge their sync onto the next PE instruction."""
    n_del = 0
    for f in nc.m.functions:
        for bb in f.blocks:
            cur = None
            pend_w, pend_u = [], []
            new = []
            for inst in bb.instructions:
                if str(inst.engine) != 'EngineType.PE':
                    new.append(inst)
                    continue
                if inst.opcode == 'Ldweights':
                    sig = (str(inst.ins), str(getattr(inst, 'perf_mode', None)),
                           str(getattr(inst, 'is_transpose', None)),
                           str(getattr(inst, 'tile_position', None)))
                    if sig == cur:
                        si = inst.sync_info
                        if si is not None:
                            pend_w.extend(si.on_wait)
                            pend_u.extend(si.on_update)
                        n_del += 1
                        continue
                    cur = sig
                    new.append(inst)
                elif inst.opcode in ('Matmult', 'NoOp'):
                    if pend_w or pend_u:
                        si = inst.sync_info
                        ow = list(si.on_wait) if si else []
                        ou = list(si.on_update) if si else []
                        inst.sync_info = mybir.SyncInfo(
                            on_wait=pend_w + ow, on_update=pend_u + ou)
                        pend_w, pend_u = [], []
                    new.append(inst)
                else:
                    cur = None
                    new.append(inst)
            assert not pend_w and not pend_u
            bb.instructions = new
    return n_del


_CACHE = {}


def _build():
    if "nc" in _CACHE:
        return _CACHE["nc"]
    nc = bass.Bass("TRN2", target_bir_lowering=False, debug=False, num_devices=8)

    # ---- I/O ----
    c8 = nc.dram_tensor("c8", [C, SLAB * 2 * Wp], F8, kind="ExternalInput")
    xv_r = nc.dram_tensor("xv_r", [C, SLAB, Wp], BF16, kind="ExternalInput")
    xv_i = nc.dram_tensor("xv_i", [C, SLAB, Wp], BF16, kind="ExternalInput")
    xv_s = nc.dram_tensor("xv_s", [C, SLAB, Wp], BF16, kind="ExternalInput")
    m8 = nc.dram_tensor("m8", [C, 9, 2, 4 * C], F8, kind="ExternalInput")
    mv_r = nc.dram_tensor("mv_r", [C, 9, C], BF16, kind="ExternalInput")
    mv_i = nc.dram_tensor("mv_i", [C, 9, C], BF16, kind="ExternalInput")
    mv_s = nc.dram_tensor("mv_s", [C, 9, C], BF16, kind="ExternalInput")
    projT_r = nc.dram_tensor("projT_r", [C, C], F32, kind="ExternalInput")
    projT_i = nc.dram_tensor("projT_i", [C, C], F32, kind="ExternalInput")
    tempv_r = nc.dram_tensor("tempv_r", [C, 1], F32, kind="ExternalInput")
    tempv_i = nc.dram_tensor("tempv_i", [C, 1], F32, kind="ExternalInput")
    out_r = nc.dram_tensor("out_r", [C, HH, W], F32, kind="ExternalOutput")
    out_i = nc.dram_tensor("out_i", [C, HH, W], F32, kind="ExternalOutput")

    cc_in = nc.dram_tensor("cc_in", [C, 516], F32)
    cc_out = nc.dram_tensor("cc_out", [C, 516], F32)

    ident_d = nc.inline_tensor(np.eye(C, dtype=np.float32), name="ident")
    ones_d = nc.inline_tensor(np.ones((1, C), dtype=np.float32), name="ones1")
    _mask = np.zeros((C, C), np.float32)
    for h in range(HEADS):
        _mask[16 * h:16 * h + 16, 16 * h:16 * h + 16] = 1.0
    off_d = nc.inline_tensor((1.0 - _mask) * -1e30, name="blkoff")

    xvr_flat = xv_r.ap().rearrange("p r c -> p (r c)")
    xvi_flat = xv_i.ap().rearrange("p r c -> p (r c)")
    xvs_flat = xv_s.ap().rearrange("p r c -> p (r c)")
    or_flat = out_r.ap().rearrange("p r c -> p (r c)")
    oi_flat = out_i.ap().rearrange("p r c -> p (r c)")

    with tile.TileContext(nc) as tc:
        with tc.tile_pool(name="persist", bufs=1) as pp:
            # persistent tiles
            m8_t = pp.tile([C, 9, 2, 4 * C], F8)
            mvr_t = pp.tile([C, 9, C], BF16)
            mvi_t = pp.tile([C, 9, C], BF16)
            mvs_t = pp.tile([C, 9, C], BF16)
            ident_t = pp.tile([C, C], F32)
            ones_t = pp.tile([1, C], F32)
            off_t = pp.tile([C, C], F32)
            pTr = pp.tile([C, C], F32)
            pTi = pp.tile([C, C], F32)
            pTin = pp.tile([C, C], F32)
            tvr = pp.tile([C, 1], F32)
            tvi = pp.tile([C, 1], F32)
            FT = pp.tile([C, 3 * C], BF16)
            ssq_acc = pp.tile([C, 4, NB], F32)
            v_rt = pp.tile([C, NPX], BF16)
            v_it = pp.tile([C, NPX], BF16)

            nc.sync.dma_start(ident_t[:], ident_d.ap())
            nc.sync.dma_start(ones_t[:], ones_d.ap())
            nc.sync.dma_start(off_t[:], off_d.ap())
            nc.sync.dma_start(pTr[:], projT_r.ap())
            nc.sync.dma_start(pTi[:], projT_i.ap())
            nc.sync.dma_start(tvr[:], tempv_r.ap())
            nc.sync.dma_start(tvi[:], tempv_i.ap())
            nc.scalar.dma_start(
                m8_t[:].rearrange("p a b c -> p (a b c)"),
                m8.ap().rearrange("p a b c -> p (a b c)"))
            nc.scalar.dma_start(
                mvr_t[:].rearrange("p a b -> p (a b)"),
                mv_r.ap().rearrange("p a b -> p (a b)"))
            nc.scalar.dma_start(
                mvi_t[:].rearrange("p a b -> p (a b)"),
                mv_i.ap().rearrange("p a b -> p (a b)"))
            nc.scalar.dma_start(
                mvs_t[:].rearrange("p a b -> p (a b)"),
                mv_s.ap().rearrange("p a b -> p (a b)"))
            nc.vector.tensor_scalar_mul(pTin[:], pTi[:], -1.0)

            # ---- phase A: fp8 DoubleRow q,k conv + Gram, streamed ----
            with tc.tile_pool(name="gram_ps", bufs=1, space="PSUM") as psg:
                gram = psg.tile([C, 512], F32)
                # 6+2 chunk sets, single-buffered PSUM: one DoubleRow
                # weight load covers up to 6 chunks (LDWEIGHTS dominates
                # DoubleRow matmul cost on HW)
                with (
                    tc.tile_pool(name="xp", bufs=2) as xp,
                    tc.tile_pool(name="yp", bufs=2) as yp,
                    tc.tile_pool(name="qkp", bufs=1) as qkp,
                    tc.tile_pool(name="sqp", bufs=1) as sqp,
                    tc.tile_pool(name="m_ps", bufs=1, space="PSUM") as psm,
                ):
                    SETS = [(0, 1, 2, 3, 4, 5), (6, 7)]
                    first_gram = [True]
                    for i in range(NB):
                        c_t = xp.tile([C, 2 * GN + 4], F8, tag="c8")
                        base = i * RB * 2 * Wp
                        nc.sync.dma_start(
                            c_t[:, 2:2 * GN + 2], c8.ap()[:, base:base + 2 * GN])

                        q_r = yp.tile([C, RB, W], BF16, tag="q_r")
                        q_i = yp.tile([C, RB, W], BF16, tag="q_i")
                        k_r = yp.tile([C, RB, W], BF16, tag="k_r")
                        k_i = yp.tile([C, RB, W], BF16, tag="k_i")
                        dsts = [[q_r, k_r], [q_i, k_i]]

                        for cs in SETS:
                            for g in range(2):
                                for part in range(2):
                                    pss = []
                                    for ci in range(len(cs)):
                                        sw_ps = psm.tile(
                                            [C, CH_N], F32, tag=f"sw{ci}",
                                            name=f"sw{ci}")
                                        pss.append(sw_ps)
                                    lsl = slice((2 * part + g) * C,
                                                (2 * part + g + 1) * C)
                                    for t in range(9):
                                        st, sp = (t == 0), (t == 8)
                                        for ci, j in enumerate(cs):
                                            cb = 1 + (2 * j + 1) * Wp
                                            off8 = 2 * (cb + (t // 3 - 1) * Wp
                                                        + (t % 3 - 1))
                                            rhs = c_t[:, off8:off8 + 2 * CH_N] \
                                                .rearrange(
                                                    "p (n two) -> p two n",
                                                    two=2)
                                            nc.tensor.matmul(
                                                pss[ci][:], m8_t[:, t, :, lsl],
                                                rhs, start=st, stop=sp,
                                                perf_mode=DR)
                                    for ci, j in enumerate(cs):
                                        yv = pss[ci][:].rearrange(
                                            "p (r c) -> p r c", r=2)
                                        rsl = slice(2 * j, 2 * j + 2)
                                        dst = dsts[part][g]
                                        if part == 0:
                                            nc.scalar.activation(
                                                dst[:, rsl, :],
                                                yv[:, :, 1:W + 1], AF.Copy)
                                        else:
                                            nc.vector.tensor_copy(
                                                dst[:, rsl, :],
                                                yv[:, :, 1:W + 1])

                        # blocked DMA-XBAR transposes (one per tensor per
                        # block: [128, 24*128] -> [128, 24, 128]) + Gram
                        flats = [q_r[:].rearrange("p r c -> p (r c)"),
                                 q_i[:].rearrange("p r c -> p (r c)"),
                                 k_r[:].rearrange("p r c -> p (r c)"),
                                 k_i[:].rearrange("p r c -> p (r c)")]
                        nch = RB * W // C  # 24 transpose chunks
                        qkT = []
                        for k4 in range(4):
                            qkT_t = qkp.tile([C, nch, C], BF16,
                                             tag=f"qkT{k4}", name=f"qkT{k4}")
                            nc.sync.dma_start(qkT_t[:], flats[k4],
                                              transpose=True)
                            qkT.append(qkT_t)
                        for cix in range(nch):
                            st = first_gram[0]
                            sp = (i == NB - 1) and (cix == nch - 1)
                            # gram layout: [qr@kr | qr@ki | qi@kr | qi@ki]
                            nc.tensor.matmul(
                                gram[:, 0:C], qkT[0][:, cix, :],
                                qkT[2][:, cix, :],
                                start=st, stop=sp, skip_group_check=True)
                            nc.tensor.matmul(
                                gram[:, C:2 * C], qkT[0][:, cix, :],
                                qkT[3][:, cix, :],
                                start=False, stop=sp, skip_group_check=True)
                            nc.tensor.matmul(
                                gram[:, 2 * C:3 * C], qkT[1][:, cix, :],
                                qkT[2][:, cix, :],
                                start=False, stop=sp, skip_group_check=True)
                            nc.tensor.matmul(
                                gram[:, 3 * C:4 * C], qkT[1][:, cix, :],
                                qkT[3][:, cix, :],
                                start=False, stop=sp, skip_group_check=True)
                            first_gram[0] = False

                        # sumsq via ACT square + accum
                        sq_t = sqp.tile([C, RB * W], BF16, tag="sq")
                        for k4 in range(4):
                            nc.scalar.activation(
                                sq_t[:], flats[k4][:], AF.Square,
                                accum_out=ssq_acc[:, k4, i:i + 1])

                # ---- A->B: stage Gram+sumsq, launch AllReduce ----
                with tc.tile_pool(name="stg", bufs=1) as stg:
                    stage = stg.tile([C, 516], F32)
                    nc.vector.tensor_copy(stage[:, 0:512], gram[:])
                    nc.vector.tensor_reduce(
                        stage[:, 512:516], ssq_acc[:],
                        axis=mybir.AxisListType.X, op=ALU.add)
                    nc.sync.dma_start(cc_in.ap(), stage[:])
                    nc.gpsimd.collective_compute(
                        "AllReduce", ALU.add,
                        replica_groups=[[0, 1], [2, 3], [4, 5], [6, 7]],
                        ins=[cc_in.ap()], outs=[cc_out.ap()])
            # gram PSUM bank freed here

            # ---- phase B: bf16 Gauss v conv into SBUF v tiles ----
            # (overlaps the AllReduce; phase C1 softmax is emitted between
            # blocks 4 and 5 so it executes inside phase B's window)
            Amats = []

            def emit_softmax(p3, ps3b):
                P = p3.tile([C, 516], F32, name="P")
                nc.sync.dma_start(P[:], cc_out.ap())

                nrm = p3.tile([C, 4], F32, name="nrm")
                nc.scalar.activation(nrm[:], P[:, 512:516], AF.Sqrt)
                rsq = p3.tile([C, 4], F32, name="rsq")
                nc.vector.reciprocal(rsq[:], nrm[:])

                prow = ps3b.tile([1, 256], F32, name="prow")
                nc.tensor.transpose(prow[0:1, 0:C], rsq[:, 2:3], ident_t[:])
                nc.tensor.transpose(prow[0:1, C:2 * C], rsq[:, 3:4], ident_t[:])
                rowb = p3.tile([1, 256], F32, name="rowb")
                nc.vector.tensor_copy(rowb[:], prow[:])
                pbc = ps3b.tile([C, 256], F32, name="pbc")
                nc.tensor.matmul(pbc[:], ones_t[:], rowb[:],
                                 start=True, stop=True)
                bc = p3.tile([C, 256], F32, name="bc")
                nc.vector.tensor_copy(bc[:], pbc[:])

                S1s = p3.tile([C, 256], F32, name="S1s")
                S2s = p3.tile([C, 256], F32, name="S2s")
                nc.vector.scalar_tensor_tensor(
                    S1s[:], P[:, 0:256], rsq[:, 0:1], bc[:],
                    op0=ALU.mult, op1=ALU.mult)
                nc.vector.scalar_tensor_tensor(
                    S2s[:], P[:, 256:512], rsq[:, 1:2], bc[:],
                    op0=ALU.mult, op1=ALU.mult)
                ar = p3.tile([C, C], F32, name="ar")
                ai = p3.tile([C, C], F32, name="ai")
                nc.vector.tensor_sub(ar[:], S1s[:, 0:C], S2s[:, C:2 * C])
                nc.vector.tensor_add(ai[:], S1s[:, C:2 * C], S2s[:, 0:C])

                for nidx, (logit, tv) in enumerate([(ar, tvr), (ai, tvi)]):
                    lg = p3.tile([C, C], F32, tag=f"lg{nidx}", name="lg")
                    nc.vector.scalar_tensor_tensor(
                        lg[:], logit[:], tv[:], off_t[:],
                        op0=ALU.mult, op1=ALU.add)
                    mx = p3.tile([C, 1], F32, tag=f"mx{nidx}", name="mx")
                    nc.vector.tensor_reduce(
                        mx[:], lg[:], axis=mybir.AxisListType.X, op=ALU.max)
                    nc.vector.tensor_scalar_sub(lg[:], lg[:], mx[:])
                    ex = p3.tile([C, C], F32, tag=f"ex{nidx}", name="ex")
                    nc.scalar.activation(ex[:], lg[:], AF.Exp)
                    sm = p3.tile([C, 1], F32, tag=f"sm{nidx}", name="sm")
                    nc.vector.tensor_reduce(
                        sm[:], ex[:], axis=mybir.AxisListType.X, op=ALU.add)
                    smi = p3.tile([C, 1], F32, tag=f"smi{nidx}", name="smi")
                    nc.vector.reciprocal(smi[:], sm[:])
                    Amat = p3.tile([C, C], F32, tag=f"Amat{nidx}", name="Amat")
                    nc.vector.tensor_scalar_mul(Amat[:], ex[:], smi[:])
                    Amats.append(Amat)

            SETS_V = [(0, 1), (2, 3), (4, 5), (6, 7)]
            p3_cm = tc.tile_pool(name="p3", bufs=1)
            p3 = p3_cm.__enter__()
            with (
                tc.tile_pool(name="vxp", bufs=2) as vxp,
                tc.tile_pool(name="sep", bufs=2) as sep,
                tc.tile_pool(name="v_ps", bufs=1, space="PSUM") as psv,
                tc.tile_pool(name="ps3b", bufs=1, space="PSUM") as ps3b,
            ):
                for i in range(NB):
                    xr_t = vxp.tile([C, GN + 2], BF16, tag="xr")
                    xi_t = vxp.tile([C, GN + 2], BF16, tag="xi")
                    xs_t = vxp.tile([C, GN + 2], BF16, tag="xs")
                    base = i * RB * Wp
                    nc.sync.dma_start(
                        xr_t[:, 1:GN + 1], xvr_flat[:, base:base + GN])
                    nc.scalar.dma_start(
                        xi_t[:, 1:GN + 1], xvi_flat[:, base:base + GN])
                    nc.sync.dma_start(
                        xs_t[:, 1:GN + 1], xvs_flat[:, base:base + GN])
                    xts = (xr_t, xi_t, xs_t)
                    for cs in SETS_V:
                        ms = []
                        for term in range(3):
                            row = []
                            for ci in range(2):
                                m_ps = psv.tile(
                                    [C, CH_N], F32, tag=f"m{term}{ci}",
                                    name=f"m{term}{ci}")
                                row.append(m_ps)
                            ms.append(row)
                        for term, mt in enumerate((mvr_t, mvi_t, mvs_t)):
                            for t in range(9):
                                st, sp = (t == 0), (t == 8)
                                for ci, j in enumerate(cs):
                                    cb = 1 + (2 * j + 1) * Wp
                                    off = cb + (t // 3 - 1) * Wp + (t % 3 - 1)
                                    nc.tensor.matmul(
                                        ms[term][ci][:], mt[:, t, :],
                                        xts[term][:, off:off + CH_N],
                                        start=st, stop=sp)
                        for ci, j in enumerate(cs):
                            m1, m2, m3 = ms[0][ci], ms[1][ci], ms[2][ci]
                            c1 = sep.tile([C, CH_N], F32, tag="c1")
                            nc.scalar.activation(c1[:], m1[:], AF.Copy)
                            s12 = sep.tile([C, CH_N], F32, tag="s12")
                            nc.vector.tensor_add(s12[:], c1[:], m2[:])
                            c1v = c1[:].rearrange("p (r c) -> p r c", r=2)
                            m2v = m2[:].rearrange("p (r c) -> p r c", r=2)
                            m3v = m3[:].rearrange("p (r c) -> p r c", r=2)
                            s12v = s12[:].rearrange("p (r c) -> p r c", r=2)
                            vo = i * RB * W + 2 * j * W
                            vrv = v_rt[:, vo:vo + 2 * W].rearrange(
                                "p (r c) -> p r c", r=2)
                            viv = v_it[:, vo:vo + 2 * W].rearrange(
                                "p (r c) -> p r c", r=2)
                            nc.vector.tensor_sub(
                                vrv, c1v[:, :, 1:W + 1], m2v[:, :, 1:W + 1])
                            nc.vector.tensor_sub(
                                viv, m3v[:, :, 1:W + 1], s12v[:, :, 1:W + 1])
                    if i == NB - 2:
                        # softmax hides inside phase B (allreduce done)
                        emit_softmax(p3, ps3b)

            # ---- phase C2: proj fusion FT = [(PA)r^T|(PA)i^T|-(PA)i^T]
            # (PA)^T = A^T P^T ; matmul(out, lhsT=Amat, rhs=pT) = A^T @ pT
            if True:
                with tc.tile_pool(name="ps3c", bufs=1, space="PSUM") as ps3c:
                    Ar_t, Ai_t = Amats
                    ftr_ps = ps3c.tile([C, C], F32, tag="ftr")
                    fti_ps = ps3c.tile([C, C], F32, tag="fti")
                    # ordered so consecutive matmuls share the loaded lhsT
                    nc.tensor.matmul(ftr_ps[:], Ar_t[:], pTr[:],
                                     start=True, stop=False)
                    nc.tensor.matmul(fti_ps[:], Ar_t[:], pTi[:],
                                     start=True, stop=False)
                    nc.tensor.matmul(ftr_ps[:], Ai_t[:], pTin[:],
                                     start=False, stop=True)
                    nc.tensor.matmul(fti_ps[:], Ai_t[:], pTr[:],
                                     start=False, stop=True)
                    nc.vector.tensor_copy(FT[:, 0:C], ftr_ps[:])
                    nc.scalar.activation(FT[:, C:2 * C], fti_ps[:], AF.Copy)
                    nc.vector.tensor_scalar_mul(
                        FT[:, 2 * C:3 * C], fti_ps[:], -1.0)
            p3_cm.__exit__(None, None, None)

            # ---- phase D: fused (P.A)@v from SBUF v, FT-outer order ----
            with (
                tc.tile_pool(name="op", bufs=2) as op_,
                tc.tile_pool(name="ps4", bufs=1, space="PSUM") as ps4,
            ):
                for k in range(NP4):
                    pb = k * P4N
                    fr = op_.tile([C, P4N], F32, tag="fr")
                    fi = op_.tile([C, P4N], F32, tag="fi")
                    prs, pis = [], []
                    for s in range(3):
                        pfr = ps4.tile([C, 512], F32, tag=f"pfr{s}",
                                       name=f"pfr{s}")
                        pfi = ps4.tile([C, 512], F32, tag=f"pfi{s}",
                                       name=f"pfi{s}")
                        prs.append(pfr)
                        pis.append(pfi)
                    sls = [slice(pb + s * 512, pb + (s + 1) * 512)
                           for s in range(3)]
                    # FT[:,0:C] pass: one weight load for 6 matmuls
                    for s in range(3):
                        nc.tensor.matmul(prs[s][:], FT[:, 0:C], v_rt[:, sls[s]],
                                         start=True, stop=False)
                        nc.tensor.matmul(pis[s][:], FT[:, 0:C], v_it[:, sls[s]],
                                         start=True, stop=False)
                    for s in range(3):
                        nc.tensor.matmul(prs[s][:], FT[:, 2 * C:3 * C],
                                         v_it[:, sls[s]],
                                         start=False, stop=True)
                    for s in range(3):
                        nc.tensor.matmul(pis[s][:], FT[:, C:2 * C],
                                         v_rt[:, sls[s]],
                                         start=False, stop=True)
                    for s in range(3):
                        osl = slice(s * 512, (s + 1) * 512)
                        nc.scalar.activation(fr[:, osl], prs[s][:], AF.Copy)
                        nc.vector.tensor_copy(fi[:, osl], pis[s][:])
                    nc.sync.dma_start(or_flat[:, pb:pb + P4N], fr[:])
                    nc.scalar.dma_start(oi_flat[:, pb:pb + P4N], fi[:])

    _dedupe_ldweights(nc)
    _split_multi_waits(nc)
    _CACHE["nc"] = nc
    return nc


def _host_inputs(x_real, x_imag, qkv_wr, qkv_wi, dw_wr, dw_wi,
                 proj_wr, proj_wi, temp_r, temp_i):
    f = np.float32
    f8 = ml_dtypes.float8_e4m3
    bf = ml_dtypes.bfloat16
    qkvT_r = np.ascontiguousarray(np.asarray(qkv_wr, f).T)
    qkvT_i = np.ascontiguousarray(np.asarray(qkv_wi, f).T)
    dwt_r = np.asarray(dw_wr, f).reshape(C3, 9).T          # [9, 384]
    dwt_i = np.asarray(dw_wi, f).reshape(C3, 9).T
    # fused complex tap matrices, [in-ch, tap, out-ch]
    m_r = qkvT_r[:, None, :] * dwt_r[None] - qkvT_i[:, None, :] * dwt_i[None]
    m_i = qkvT_r[:, None, :] * dwt_i[None] + qkvT_i[:, None, :] * dwt_r[None]

    # fp8 DoubleRow packs for q,k (scaled; scale cancels in normalization)
    s = M8SCALE
    m8 = np.empty((C, 9, 2, 4 * C), f8)
    qk = slice(0, 2 * C)
    m8[:, :, 0, 0:2 * C] = (s * m_r[:, :, qk]).astype(f8)
    m8[:, :, 1, 0:2 * C] = (-s * m_i[:, :, qk]).astype(f8)
    m8[:, :, 0, 2 * C:4 * C] = (s * m_i[:, :, qk]).astype(f8)
    m8[:, :, 1, 2 * C:4 * C] = (s * m_r[:, :, qk]).astype(f8)

    # bf16 Gauss taps for v
    vc = slice(2 * C, 3 * C)
    mv_r = np.ascontiguousarray(m_r[:, :, vc]).astype(bf)
    mv_i = np.ascontiguousarray(m_i[:, :, vc]).astype(bf)
    mv_s = np.ascontiguousarray(m_r[:, :, vc] + m_i[:, :, vc]).astype(bf)

    projT_r = np.ascontiguousarray(np.asarray(proj_wr, f).T)
    projT_i = np.ascontiguousarray(np.asarray(proj_wi, f).T)
    tvr = np.repeat(np.asarray(temp_r, f).reshape(HEADS), 16).reshape(C, 1)
    tvi = np.repeat(np.asarray(temp_i, f).reshape(HEADS), 16).reshape(C, 1)
    tvr = np.ascontiguousarray(tvr)
    tvi = np.ascontiguousarray(tvi)

    xr = np.asarray(x_real, f)
    xi = np.asarray(x_imag, f)
    in_maps = []
    for core in range(8):
        b, hh = core // 2, core % 2
        lo = hh * HH - 1
        sl_r = np.zeros((C, SLAB, Wp), f)
        sl_i = np.zeros((C, SLAB, Wp), f)
        s0 = max(lo, 0)
        s1 = min(lo + SLAB, H)
        d0 = s0 - lo
        sl_r[:, d0:d0 + (s1 - s0), 1:W + 1] = xr[b, :, s0:s1, :]
        sl_i[:, d0:d0 + (s1 - s0), 1:W + 1] = xi[b, :, s0:s1, :]
        c8a = np.empty((C, SLAB, Wp, 2), f8)
        c8a[..., 0] = sl_r.astype(f8)
        c8a[..., 1] = sl_i.astype(f8)
        in_maps.append({
            "c8": c8a.reshape(C, SLAB * 2 * Wp),
            "xv_r": sl_r.astype(bf), "xv_i": sl_i.astype(bf),
            "xv_s": (sl_r + sl_i).astype(bf),
            "m8": m8, "mv_r": mv_r, "mv_i": mv_i, "mv_s": mv_s,
            "projT_r": projT_r, "projT_i": projT_i,
            "tempv_r": tvr, "tempv_i": tvi,
        })
    return in_maps


def kernel(**inputs):
    nc = _build()
    in_maps = _host_inputs(**inputs)
    res = run_bass_kernel_spmd(nc, in_maps, list(range(8)))
    out_r = np.empty((B, C, H, W), np.float32)
    out_i = np.empty((B, C, H, W), np.float32)
    for core in range(8):
        b, hh = core // 2, core % 2
        out_r[b, :, hh * HH:(hh + 1) * HH, :] = res.results[core]["out_r"]
        out_i[b, :, hh * HH:(hh + 1) * HH, :] = res.results[core]["out_i"]
    return out_r, out_i
